# revision 11
# baseline (speedup 1.0000x reference)
"""Trainium2 Bass kernel for a pre-norm cross-attention transformer layer.

Reference computation (B=4, Lq=Lk=1024, E=1024, H=16, Dh=64, F=4096):
    t2 = LN(tgt); q = t2@wq+bq; k = mem@wk+bk; v = mem@wv+bv
    p = softmax(q k^T / sqrt(Dh)); attn = mean_h(p)
    x = tgt + (p v)@wo + bo
    out = x + relu(LN(x)@w1+b1)@w2 + b2
Returns (out, attn).

Sharding: 8 cores = 4 batches x 2 query-halves. Each core owns 512 query rows
of one batch, computes K/V for its batch's full memory (duplicated within the
pair), and produces disjoint slices of both outputs -> no collectives.

On-device layout: all activations are kept transposed ([features, rows]) so
every matmul's contraction dim sits on SBUF partitions. The host passes
pre-transposed inputs and un-transposes outputs, so no device-side layout
shuffles are needed. Matmul operands are bf16 (fp32 matmul is 4x slower);
accumulation is fp32 in PSUM; softmax/LN bookkeeping is fp32.

Softmax denominators come for free: wv is host-augmented with one extra
all-zero column per head whose bias row is 1, so the p@v accumulation's 65th
output row is sum_k exp(score). 1/denom is computed as exp(-ln(d)) because Ln
and Exp share one ACT table set (Rsqrt/Reciprocal are banned/inaccurate).
"""

import math
import os
import sys
from contextlib import ExitStack

for _p in ("/opt/trn_rl_repo", "/root/.axon_site/_ro/trn_rl_repo"):
    if os.path.isdir(_p) and _p not in sys.path:
        sys.path.append(_p)

import ml_dtypes
import numpy as np

import concourse.bass as bass
import concourse.tile as tile
from concourse import mybir
from concourse.bass_utils import run_bass_kernel_spmd
from concourse.vector_clock import ScopedClock

F32 = mybir.dt.float32
BF16 = mybir.dt.bfloat16
AF = mybir.ActivationFunctionType
OP = mybir.AluOpType
BF = ml_dtypes.bfloat16

B, LQ, LK, E, H, F = 4, 1024, 1024, 1024, 16, 4096
DH = E // H          # 64
R = 512              # query rows per core
SCALE = 1.0 / math.sqrt(DH)
HW = DH + 1          # head width in augmented v (64 dims + denom ones col)
N_CORES = 8
EPS = 1e-5


class PatchedTileContext(tile.TileContext):
    """Splits the kernel-tail drain's semaphore waits into individual wait_ge
    instructions; the installed walrus rejects >2 sync waits per instruction."""

    def _drain_and_barrier(self, tick_clock, wait_clock):
        nc = self.nc
        nop_inst = nc.sync.nop()
        wait_clock.add_sem_waits(
            nop_inst.ins, ScopedClock({None: tick_clock.global_clock})
        )
        mi = nop_inst.ins
        waits = list(mi.sync_info.on_wait) if (mi.sync_info and mi.sync_info.on_wait) else []
        if mi.sync_info is not None:
            mi.sync_info.on_wait = []
        assert self.sems is not None
        sem_by_id = {s.num: s for s in self.sems.allocated().values()}
        for w in waits:
            sem = sem_by_id.get(w.id)
            assert sem is not None, f"no sem handle for wait {w}"
            nc.sync.wait_ge(sem, w.wait_value)
        nc.sync.drain()

        nc.all_engine_barrier()
        popped = nc._tile_sem_poison_stack.pop()
        assert popped is self._sem_poison
        nc.clear_and_free_semaphores(list(self.sems.allocated().values()))
        nc.all_engine_barrier()


def _emit_layernorm_T(nc, xT, g_t, b_t, outT, ones_col_f32, ones_col_bf,
                      ones_row_f32, sq_pool, tmp_pool, stat_pool, bcast_pool,
                      small_pool, eps_t):
    """LayerNorm over features of a transposed activation.

    xT:   SBUF [128, 8, 512] f32   (feature-major; feature f = 128*t + p)
    outT: SBUF [128, 8, 512] bf16  normalized * g + b
    Row stats come from ones-vector matmuls (partition+tile reduction in one
    PSUM chain); A=rstd / B=-mean*rstd are broadcast to 128 partitions with a
    rank-1 ones matmul and applied as (x*A + B) * g + b.  The [1,512] scratch
    vectors rotate through 4 pool slots (a [1,N] tile still costs N bytes on
    every partition, so slots are kept few).
    """
    ps_sum = stat_pool.tile([1, R], F32, tag="st_sum", name="ps_sum")
    ps_sq = stat_pool.tile([1, R], F32, tag="st_sq", name="ps_sq")
    for ti in range(8):
        sq_t = sq_pool.tile([128, R], BF16, tag="sq", name="sq_t")
        nc.scalar.activation(out=sq_t[:], in_=xT[:, ti, :], func=AF.Square)
        nc.tensor.matmul(ps_sum[:], ones_col_f32[:], xT[:, ti, :],
                         start=(ti == 0), stop=(ti == 7))
        nc.tensor.matmul(ps_sq[:], ones_col_bf[:], sq_t[:],
                         start=(ti == 0), stop=(ti == 7))
    mean = small_pool.tile([1, R], F32, tag="s0", name="mean")
    nc.vector.tensor_single_scalar(out=mean[:], in_=ps_sum[:], scalar=1.0 / E, op=OP.mult)
    ex2 = small_pool.tile([1, R], F32, tag="s1", name="ex2")
    nc.vector.tensor_single_scalar(out=ex2[:], in_=ps_sq[:], scalar=1.0 / E, op=OP.mult)
    msq = small_pool.tile([1, R], F32, tag="s2", name="msq")
    nc.vector.tensor_mul(out=msq[:], in0=mean[:], in1=mean[:])
    var = small_pool.tile([1, R], F32, tag="s3", name="var")
    nc.vector.tensor_sub(out=var[:], in0=ex2[:], in1=msq[:])
    lnv = small_pool.tile([1, R], F32, tag="s1", name="lnv")
    nc.scalar.activation(out=lnv[:], in_=var[:], func=AF.Ln, bias=eps_t[0:1, :])
    A = small_pool.tile([1, R], F32, tag="s2", name="A")
    nc.scalar.activation(out=A[:], in_=lnv[:], func=AF.Exp, scale=-0.5)
    Bv = small_pool.tile([1, R], F32, tag="s3", name="Bv")
    nc.vector.scalar_tensor_tensor(out=Bv[:], in0=mean[:], scalar=-1.0, in1=A[:],
                                   op0=OP.mult, op1=OP.mult)
    ps_A = bcast_pool.tile([128, R], F32, tag="bc_A", name="ps_A")
    ps_B = bcast_pool.tile([128, R], F32, tag="bc_B", name="ps_B")
    nc.tensor.matmul(ps_A[:], ones_row_f32[:], A[:], start=True, stop=True)
    nc.tensor.matmul(ps_B[:], ones_row_f32[:], Bv[:], start=True, stop=True)
    for ti in range(8):
        tmp = tmp_pool.tile([128, R], F32, tag="ln_tmp", name="tmp")
        nc.vector.tensor_mul(out=tmp[:], in0=xT[:, ti, :], in1=ps_A[:])
        tmp2 = tmp_pool.tile([128, R], F32, tag="ln_tmp2", name="tmp2")
        nc.vector.tensor_add(out=tmp2[:], in0=tmp[:], in1=ps_B[:])
        nc.scalar.activation(out=outT[:, ti, :], in_=tmp2[:], func=AF.Identity,
                             scale=g_t[:, ti:ti + 1], bias=b_t[:, ti:ti + 1])


def _split_sync_waits(nc, maxw=1):
    """Walrus rejects instructions carrying more than a couple of sync waits
    ("Too many sync wait commands"). Move excess waits onto NoOp instructions
    inserted just before, on the same engine queue — semantically identical
    (the engine blocks on the NoOp's wait before reaching the instruction)."""
    cnt = 0
    for f in nc.m.functions:
        for bb in f.blocks:
            insts = bb.instructions
            new_list = []
            for inst in insts:
                si = inst.sync_info
                waits = list(si.on_wait) if (si and si.on_wait) else []
                if len(waits) > maxw:
                    extra, keep = waits[:-maxw], waits[-maxw:]
                    si.on_wait = keep
                    for w in extra:
                        cnt += 1
                        nop = mybir.InstNoOp(
                            name=f"waitsplit-{cnt}", ins=[], outs=[],
                            engine=inst.engine,
                            sync_info=mybir.SyncInfo(on_wait=[w], on_update=[]))
                        new_list.append(nop)
                new_list.append(inst)
            insts[:] = new_list
    return cnt


def build_program():
    nc = bass.Bass("TRN2", target_bir_lowering=False, debug=False,
                   num_devices=N_CORES)

    # ---- DRAM parameters (per-core views, host-prepped) ----
    d_tgtT = nc.declare_dram_parameter("tgtT", [E, R], F32, isOutput=False)
    d_memT = nc.declare_dram_parameter("memT", [E, LK], BF16, isOutput=False)
    d_wq = nc.declare_dram_parameter("wq", [E, E], BF16, isOutput=False)
    d_wk = nc.declare_dram_parameter("wk", [E, E], BF16, isOutput=False)
    d_wva = nc.declare_dram_parameter("wva", [E, H * HW], BF16, isOutput=False)
    d_wo = nc.declare_dram_parameter("wo", [E, E], BF16, isOutput=False)
    d_w1 = nc.declare_dram_parameter("w1", [E, F], BF16, isOutput=False)
    d_w2 = nc.declare_dram_parameter("w2", [F, E], BF16, isOutput=False)
    d_bq = nc.declare_dram_parameter("bqp", [128, 8], F32, isOutput=False)
    d_bk = nc.declare_dram_parameter("bkp", [128, 8], F32, isOutput=False)
    d_bva = nc.declare_dram_parameter("bvap", [1, H * HW], BF16, isOutput=False)
    d_bo = nc.declare_dram_parameter("bop", [128, 8], F32, isOutput=False)
    d_b1 = nc.declare_dram_parameter("b1p", [128, 32], F32, isOutput=False)
    d_b2 = nc.declare_dram_parameter("b2p", [128, 8], F32, isOutput=False)
    d_g1 = nc.declare_dram_parameter("g1p", [128, 8], F32, isOutput=False)
    d_bb1 = nc.declare_dram_parameter("bb1p", [128, 8], F32, isOutput=False)
    d_g3 = nc.declare_dram_parameter("g3p", [128, 8], F32, isOutput=False)
    d_bb3 = nc.declare_dram_parameter("bb3p", [128, 8], F32, isOutput=False)
    d_outT = nc.declare_dram_parameter("outT", [E, R], F32, isOutput=True)
    d_attnT = nc.declare_dram_parameter("attnT", [LK, R], F32, isOutput=True)

    with PatchedTileContext(nc) as tc, ExitStack() as top:
        consts = top.enter_context(tc.tile_pool(name="consts", bufs=1))
        persist = top.enter_context(tc.tile_pool(name="persist", bufs=1))

        # ---- constants / small parameter tiles ----
        ones_col_f32 = consts.tile([128, 1], F32)
        nc.vector.memset(ones_col_f32[:], 1.0)
        ones_col_bf = consts.tile([128, 1], BF16)
        nc.vector.memset(ones_col_bf[:], 1.0)
        ones_row_f32 = consts.tile([1, 128], F32)
        nc.vector.memset(ones_row_f32[:], 1.0)
        ones_sq_bf = consts.tile([128, 128], BF16)
        nc.vector.memset(ones_sq_bf[:], 1.0)
        eps_t = consts.tile([128, 1], F32)
        nc.vector.memset(eps_t[:], EPS)
        mln16_t = consts.tile([128, 1], F32)
        nc.vector.memset(mln16_t[:], -math.log(16.0))

        bq_t = consts.tile([128, 8], F32)
        bk_t = consts.tile([128, 8], F32)
        bo_t = consts.tile([128, 8], F32)
        b1_t = consts.tile([128, 32], F32)
        b2_t = consts.tile([128, 8], F32)
        g1_t = consts.tile([128, 8], F32)
        bb1_t = consts.tile([128, 8], F32)
        g3_t = consts.tile([128, 8], F32)
        bb3_t = consts.tile([128, 8], F32)
        bva_t = consts.tile([1, H * HW], BF16)
        for dst, src in ((bq_t, d_bq), (bk_t, d_bk), (bo_t, d_bo),
                         (b1_t, d_b1), (b2_t, d_b2), (g1_t, d_g1),
                         (bb1_t, d_bb1), (g3_t, d_g3), (bb3_t, d_bb3),
                         (bva_t, d_bva)):
            nc.gpsimd.dma_start(out=dst[:], in_=src[:])

        # xT spans phases 3-4; tgtT/attn_oT are freed after phase 3; the
        # attention tensors (qT/kT/v/acc/exp) after phase 2.
        xT = persist.tile([128, 8, R], F32)
        resid = ExitStack()
        residp = resid.enter_context(tc.tile_pool(name="residp", bufs=1))
        tgtT = residp.tile([128, 8, R], F32, tag="tgtT", name="tgtT")
        _tgt_src = d_tgtT[:].rearrange("(t p) r -> p t r", p=128)
        for _c in range(4):
            nc.sync.dma_start(out=tgtT[:, 2 * _c:2 * _c + 2, :],
                              in_=_tgt_src[:, 2 * _c:2 * _c + 2, :])
        attn_oT = residp.tile([128, 8, R], BF16, tag="attn_oT", name="attn_oT")

        # wo spans phases 2-3; opened before actp so pool releases stay LIFO
        wo_stack = ExitStack()
        wop = wo_stack.enter_context(tc.tile_pool(name="wop", bufs=1))
        wo_t = wop.tile([128, 8, E], BF16, tag="wo", name="wo_t")
        nc.scalar.dma_start(out=wo_t[:], in_=d_wo[:].rearrange("(t p) o -> p t o", p=128))

        attn_stack = ExitStack()
        actp = attn_stack.enter_context(tc.tile_pool(name="actp", bufs=1))

        # =========== Phase 1: LN1 + Q/K/V projections ===========
        qkv = ExitStack()
        with qkv:
            memp = qkv.enter_context(tc.tile_pool(name="memp", bufs=1))
            t2p = qkv.enter_context(tc.tile_pool(name="t2p", bufs=1))
            sqp = qkv.enter_context(tc.tile_pool(name="sqp", bufs=2))
            tmpp = qkv.enter_context(tc.tile_pool(name="tmpp", bufs=2))
            smallp = qkv.enter_context(tc.tile_pool(name="smallp", bufs=1))
            statp = qkv.enter_context(tc.tile_pool(name="statp", bufs=1, space="PSUM"))
            bcp = qkv.enter_context(tc.tile_pool(name="bcp", bufs=1, space="PSUM"))
            projp = qkv.enter_context(tc.tile_pool(name="projp", bufs=4, space="PSUM"))

            memT = memp.tile([128, 8, LK], BF16)
            nc.scalar.dma_start(out=memT[:], in_=d_memT[:].rearrange("(t p) k -> p t k", p=128))

            t2T = t2p.tile([128, 8, R], BF16)
            _emit_layernorm_T(nc, tgtT, g1_t, bb1_t, t2T, ones_col_f32,
                              ones_col_bf, ones_row_f32, sqp, tmpp, statp,
                              bcp, smallp, eps_t)

            qT = actp.tile([128, 8, R], BF16, tag="qT", name="qT")
            kT = actp.tile([128, 8, LK], BF16, tag="kT", name="kT")
            v_t = actp.tile([128, 8, H * HW], BF16, tag="vT", name="v_t")

            # wq/wk share a pool freed before wva loads (SBUF headroom)
            with tc.tile_pool(name="wqk", bufs=1) as wqk:
                wq_t = wqk.tile([128, 8, E], BF16, tag="wq", name="wq_t")
                nc.sync.dma_start(out=wq_t[:], in_=d_wq[:].rearrange("(t p) o -> p t o", p=128))
                wk_t = wqk.tile([128, 8, E], BF16, tag="wk", name="wk_t")
                nc.scalar.dma_start(out=wk_t[:], in_=d_wk[:].rearrange("(t p) o -> p t o", p=128))

                # qT[f, r] (bf16), per 128-feature output tile
                for to in range(8):
                    ps = projp.tile([128, R], F32, tag="proj", name="ps")
                    for ti in range(8):
                        nc.tensor.matmul(ps[:], wq_t[:, ti, to * 128:(to + 1) * 128],
                                         t2T[:, ti, :], start=(ti == 0), stop=(ti == 7))
                    nc.scalar.activation(out=qT[:, to, :], in_=ps[:], func=AF.Identity,
                                         bias=bq_t[:, to:to + 1])

                # kT[f, key] (bf16)
                for to in range(8):
                    for kc in range(2):
                        ps = projp.tile([128, R], F32, tag="proj", name="ps")
                        for ti in range(8):
                            nc.tensor.matmul(ps[:], wk_t[:, ti, to * 128:(to + 1) * 128],
                                             memT[:, ti, kc * 512:(kc + 1) * 512],
                                             start=(ti == 0), stop=(ti == 7))
                        nc.scalar.activation(out=kT[:, to, kc * 512:(kc + 1) * 512],
                                             in_=ps[:], func=AF.Identity,
                                             bias=bk_t[:, to:to + 1])

            # v_aug[key, hw] natural layout (bf16), 4 chunks of 260 cols
            with tc.tile_pool(name="wvap", bufs=1) as wvap:
                wva_t = wvap.tile([128, 8, H * HW], BF16, tag="wva", name="wva_t")
                nc.scalar.dma_start(out=wva_t[:], in_=d_wva[:].rearrange("(t p) o -> p t o", p=128))
                CHUNKS = ((0, 512), (512, 512), (1024, 16))
                for c0, cw in CHUNKS:
                    for kt in range(8):
                        ps = projp.tile([128, cw], F32, tag="proj", name="ps")
                        for ti in range(8):
                            nc.tensor.matmul(ps[:], memT[:, ti, kt * 128:(kt + 1) * 128],
                                             wva_t[:, ti, c0:c0 + cw],
                                             start=(ti == 0), stop=False)
                        nc.tensor.matmul(ps[:], ones_sq_bf[0:1, :],
                                         bva_t[:, c0:c0 + cw],
                                         start=False, stop=True)
                        nc.vector.tensor_copy(out=v_t[:, kt, c0:c0 + cw], in_=ps[:])

        # =========== Phase 2: attention (16 heads) ===========
        att = ExitStack()
        with att:
            accp = att.enter_context(tc.tile_pool(name="accp", bufs=1))
            acc = [accp.tile([128, R], BF16, tag=f"acc{kt}", name=f"acc{kt}") for kt in range(8)]
            expp = att.enter_context(tc.tile_pool(name="expp", bufs=10))
            cbp = att.enter_context(tc.tile_pool(name="cbp", bufs=3))
            dnp = att.enter_context(tc.tile_pool(name="dnp", bufs=3))
            odtp = att.enter_context(tc.tile_pool(name="odtp", bufs=2))
            scps = att.enter_context(tc.tile_pool(name="scps", bufs=2, space="PSUM"))
            outps = att.enter_context(tc.tile_pool(name="outps", bufs=2, space="PSUM"))
            cbps = att.enter_context(tc.tile_pool(name="cbps", bufs=2, space="PSUM"))

            for h in range(H):
                ti, off = h // 2, (h % 2) * 64
                exp_ts = []
                for ktp in range(4):
                    s_ps = scps.tile([128, 2 * R], F32, tag="sc", name="s_ps")
                    for j in range(2):
                        kt = 2 * ktp + j
                        nc.tensor.matmul(
                            s_ps[:, j * R:(j + 1) * R],
                            kT[off:off + 64, ti, kt * 128:(kt + 1) * 128],
                            qT[off:off + 64, ti, :],
                            start=True, stop=True)
                    e_t = expp.tile([128, 2 * R], BF16, tag="exp", name="e_t")
                    nc.scalar.activation(out=e_t[:], in_=s_ps[:], func=AF.Exp,
                                         scale=SCALE)
                    exp_ts.append(e_t)

                o_ps = outps.tile([128, R], F32, tag="o", name="o_ps")
                for kt in range(8):
                    nc.tensor.matmul(o_ps[0:HW, :],
                                     v_t[:, kt, h * HW:(h + 1) * HW],
                                     exp_ts[kt // 2][:, (kt % 2) * R:(kt % 2 + 1) * R],
                                     start=(kt == 0), stop=(kt == 7))

                # c = 1/(16*denom) via ln->exp (denom lives on partition 64)
                dn_t = dnp.tile([128, R], F32, tag="dn", name="dn_t")
                nc.scalar.activation(out=dn_t[64:65, :], in_=o_ps[64:65, :], func=AF.Ln)
                c_t = dnp.tile([128, R], BF16, tag="c", name="c_t")
                nc.scalar.activation(out=c_t[64:65, :], in_=dn_t[64:65, :], func=AF.Exp,
                                     scale=-1.0, bias=mln16_t[64:65, :])
                cb_ps = cbps.tile([128, R], F32, tag="cb", name="cb_ps")
                nc.tensor.matmul(cb_ps[:], ones_sq_bf[64:65, 0:128], c_t[64:65, :],
                                 start=True, stop=True)
                cb = cbp.tile([128, R], BF16, tag="cbt", name="cb")
                nc.vector.tensor_copy(out=cb[:], in_=cb_ps[:])

                # normalized per-head attention output rows (x16 undoes the /16 in c)
                if off == 0:
                    nc.vector.scalar_tensor_tensor(
                        out=attn_oT[0:64, ti, :], in0=o_ps[0:64, :], scalar=16.0,
                        in1=cb[0:64, :], op0=OP.mult, op1=OP.mult)
                else:
                    od_t = odtp.tile([64, R], BF16, tag="od", name="od_t")
                    nc.vector.scalar_tensor_tensor(
                        out=od_t[:], in0=o_ps[0:64, :], scalar=16.0,
                        in1=cb[0:64, :], op0=OP.mult, op1=OP.mult)
                    nc.gpsimd.dma_start(out=attn_oT[64:128, ti, :], in_=od_t[:])

                # head-mean accumulation of attention probabilities
                # (kt 0-4 on DVE, 5-7 on the otherwise-idle GpSimd)
                for kt in range(8):
                    eng = nc.vector if kt < 5 else nc.gpsimd
                    e_sl = exp_ts[kt // 2][:, (kt % 2) * R:(kt % 2 + 1) * R]
                    if h == 0:
                        eng.tensor_mul(out=acc[kt][:], in0=e_sl, in1=cb[:])
                    else:
                        tag = "acctmp" if kt < 5 else "acctmpg"
                        tmp = cbp.tile([128, R], BF16, tag=tag, name="tmp")
                        eng.tensor_mul(out=tmp[:], in0=e_sl, in1=cb[:])
                        eng.tensor_add(out=acc[kt][:], in0=acc[kt][:], in1=tmp[:])

            # attn output store (bf16 -> f32 cast in DMA)
            for kt in range(8):
                nc.gpsimd.dma_start(out=d_attnT[kt * 128:(kt + 1) * 128, :], in_=acc[kt][:])

        attn_stack.close()

        # =========== Phase 3: out-proj + residual ===========
        with tc.tile_pool(name="ops", bufs=2, space="PSUM") as opsp:
            for to in range(8):
                ps = opsp.tile([128, R], F32, tag="op", name="ps")
                for ti in range(8):
                    nc.tensor.matmul(ps[:], wo_t[:, ti, to * 128:(to + 1) * 128],
                                     attn_oT[:, ti, :], start=(ti == 0), stop=(ti == 7))
                nc.vector.scalar_tensor_tensor(
                    out=xT[:, to, :], in0=ps[:], scalar=bo_t[:, to:to + 1],
                    in1=tgtT[:, to, :], op0=OP.add, op1=OP.add)

        wo_stack.close()
        resid.close()

        # =========== Phase 4: LN3 + FFN ===========
        t3_stack = ExitStack()
        t3p = t3_stack.enter_context(tc.tile_pool(name="t3p", bufs=1))
        t3T = t3p.tile([128, 8, R], BF16)
        ffn = ExitStack()
        with ffn:
            w1p = ffn.enter_context(tc.tile_pool(name="w1p", bufs=2))
            w2p = ffn.enter_context(tc.tile_pool(name="w2p", bufs=1))
            htp = ffn.enter_context(tc.tile_pool(name="htp", bufs=1))
            fout = ffn.enter_context(tc.tile_pool(name="fout", bufs=3))
            ffnp = ffn.enter_context(tc.tile_pool(name="ffnp", bufs=4, space="PSUM"))

            # w2 rides the ACT HWDGE ring so the w1 chunk loads (SP ring)
            # are not queued behind this 8MB transfer
            w2_t = w2p.tile([128, 32, E], BF16, tag="w2", name="w2_t")
            nc.scalar.dma_start(out=w2_t[:], in_=d_w2[:].rearrange("(t p) o -> p t o", p=128))

            ln3 = ExitStack()
            with ln3:
                sqp4 = ln3.enter_context(tc.tile_pool(name="sqp4", bufs=2))
                tmpp4 = ln3.enter_context(tc.tile_pool(name="tmpp4", bufs=2))
                smallp4 = ln3.enter_context(tc.tile_pool(name="smallp4", bufs=1))
                statp4 = ln3.enter_context(tc.tile_pool(name="statp4", bufs=1, space="PSUM"))
                bcp4 = ln3.enter_context(tc.tile_pool(name="bcp4", bufs=1, space="PSUM"))
                _emit_layernorm_T(nc, xT, g3_t, bb3_t, t3T, ones_col_f32,
                                  ones_col_bf, ones_row_f32, sqp4, tmpp4, statp4,
                                  bcp4, smallp4, eps_t)

            hT = htp.tile([128, 32, R], BF16)
            for fog in range(8):
                w1c = w1p.tile([128, 8, R], BF16, tag="w1c", name="w1c")
                src = d_w1[:, fog * 512:(fog + 1) * 512]
                nc.sync.dma_start(out=w1c[:], in_=src.rearrange("(t p) f -> p t f", p=128))
                for f2 in range(4):
                    fo = fog * 4 + f2
                    ps = ffnp.tile([128, R], F32, tag="ffn", name="ps")
                    for ti in range(8):
                        nc.tensor.matmul(ps[:], w1c[:, ti, f2 * 128:(f2 + 1) * 128],
                                         t3T[:, ti, :], start=(ti == 0), stop=(ti == 7))
                    nc.scalar.activation(out=hT[:, fo, :], in_=ps[:], func=AF.Relu,
                                         bias=b1_t[:, fo:fo + 1])

            for eo in range(8):
                ps = ffnp.tile([128, R], F32, tag="ffn", name="ps")
                for fi in range(32):
                    nc.tensor.matmul(ps[:], w2_t[:, fi, eo * 128:(eo + 1) * 128],
                                     hT[:, fi, :], start=(fi == 0), stop=(fi == 31))
                fo_t = fout.tile([128, R], F32, tag="fo", name="fo_t")
                nc.vector.scalar_tensor_tensor(
                    out=fo_t[:], in0=ps[:], scalar=b2_t[:, eo:eo + 1],
                    in1=xT[:, eo, :], op0=OP.add, op1=OP.add)
                nc.sync.dma_start(out=d_outT[eo * 128:(eo + 1) * 128, :], in_=fo_t[:])
        t3_stack.close()
    _split_sync_waits(nc, maxw=1)
    return nc


_NC = None


def _get_program():
    global _NC
    if _NC is None:
        _NC = build_program()
    return _NC


def kernel(tgt, memory, ln1_g, ln1_b, wq, bq, wk, bk, wv, bv, wo, bo,
           ln3_g, ln3_b, w1, b1, w2, b2):
    tgt = np.asarray(tgt, np.float32)
    memory = np.asarray(memory, np.float32)

    def part_tiles(vec, n):
        # [n*128] bias -> [128, n] per-partition tiles (feature f = 128*t + p)
        return np.ascontiguousarray(np.asarray(vec, np.float32).reshape(n, 128).T)

    wq_b = np.ascontiguousarray(np.asarray(wq, np.float32)).astype(BF)
    wk_b = np.ascontiguousarray(np.asarray(wk, np.float32)).astype(BF)
    wo_b = np.ascontiguousarray(np.asarray(wo, np.float32)).astype(BF)
    w1_b = np.ascontiguousarray(np.asarray(w1, np.float32)).astype(BF)
    w2_b = np.ascontiguousarray(np.asarray(w2, np.float32)).astype(BF)
    # augmented v-projection: per head 64 value cols + 1 zero col whose bias is 1
    wva = np.zeros((E, H * HW), np.float32)
    bva = np.zeros((1, H * HW), np.float32)
    wv_f = np.asarray(wv, np.float32)
    bv_f = np.asarray(bv, np.float32)
    for h in range(H):
        wva[:, h * HW:h * HW + DH] = wv_f[:, h * DH:(h + 1) * DH]
        bva[0, h * HW:h * HW + DH] = bv_f[h * DH:(h + 1) * DH]
        bva[0, h * HW + DH] = 1.0
    wva_b = wva.astype(BF)
    bva_b = bva.astype(BF)

    shared = {
        "wq": wq_b, "wk": wk_b, "wva": wva_b, "wo": wo_b,
        "w1": w1_b, "w2": w2_b,
        "bqp": part_tiles(bq, 8), "bkp": part_tiles(bk, 8),
        "bvap": bva_b, "bop": part_tiles(bo, 8),
        "b1p": part_tiles(b1, 32), "b2p": part_tiles(b2, 8),
        "g1p": part_tiles(ln1_g, 8), "bb1p": part_tiles(ln1_b, 8),
        "g3p": part_tiles(ln3_g, 8), "bb3p": part_tiles(ln3_b, 8),
    }
    in_maps = []
    for c in range(N_CORES):
        b, hh = c // 2, c % 2
        rows = tgt[b, hh * R:(hh + 1) * R]            # [512, 1024]
        m = {"tgtT": np.ascontiguousarray(rows.T),
             "memT": np.ascontiguousarray(memory[b].T.astype(BF))}
        m.update(shared)
        in_maps.append(m)

    nc = _get_program()
    res = run_bass_kernel_spmd(nc, in_maps, list(range(N_CORES)))

    out = np.empty((B, LQ, E), np.float32)
    attn = np.empty((B, LQ, LK), np.float32)
    for c in range(N_CORES):
        b, hh = c // 2, c % 2
        out[b, hh * R:(hh + 1) * R] = res.results[c]["outT"].T
        attn[b, hh * R:(hh + 1) * R] = res.results[c]["attnT"].T
    return out, attn


# revision 13
# speedup vs baseline: 1.0700x; 1.0700x over previous
"""Trainium2 Bass kernel for a pre-norm cross-attention transformer layer.

Reference computation (B=4, Lq=Lk=1024, E=1024, H=16, Dh=64, F=4096):
    t2 = LN(tgt); q = t2@wq+bq; k = mem@wk+bk; v = mem@wv+bv
    p = softmax(q k^T / sqrt(Dh)); attn = mean_h(p)
    x = tgt + (p v)@wo + bo
    out = x + relu(LN(x)@w1+b1)@w2 + b2
Returns (out, attn).

Sharding: 8 cores = 4 batches x 2 query-halves. Each core owns 512 query rows
of one batch, computes K/V for its batch's full memory (duplicated within the
pair), and produces disjoint slices of both outputs -> no collectives.

On-device layout: all activations are kept transposed ([features, rows]) so
every matmul's contraction dim sits on SBUF partitions. The host passes
pre-transposed inputs and un-transposes outputs, so no device-side layout
shuffles are needed. Matmul operands are bf16 (fp32 matmul is 4x slower);
accumulation is fp32 in PSUM; softmax/LN bookkeeping is fp32.

Softmax denominators come for free: wv is host-augmented with one extra
all-zero column per head whose bias row is 1, so the p@v accumulation's 65th
output row is sum_k exp(score). 1/denom is computed as exp(-ln(d)) because Ln
and Exp share one ACT table set (Rsqrt/Reciprocal are banned/inaccurate).
"""

import math
import os
import sys
from contextlib import ExitStack

for _p in ("/opt/trn_rl_repo", "/root/.axon_site/_ro/trn_rl_repo"):
    if os.path.isdir(_p) and _p not in sys.path:
        sys.path.append(_p)

import ml_dtypes
import numpy as np

import concourse.bass as bass
import concourse.tile as tile
from concourse import mybir
from concourse.bass_utils import run_bass_kernel_spmd
from concourse.vector_clock import ScopedClock
from concourse.tile import add_dep_helper

F32 = mybir.dt.float32
BF16 = mybir.dt.bfloat16
AF = mybir.ActivationFunctionType
OP = mybir.AluOpType
BF = ml_dtypes.bfloat16

B, LQ, LK, E, H, F = 4, 1024, 1024, 1024, 16, 4096
DH = E // H          # 64
R = 512              # query rows per core
SCALE = 1.0 / math.sqrt(DH)
HW = DH + 1          # head width in augmented v (64 dims + denom ones col)
N_CORES = 8
EPS = 1e-5


class PatchedTileContext(tile.TileContext):
    """Splits the kernel-tail drain's semaphore waits into individual wait_ge
    instructions; the installed walrus rejects >2 sync waits per instruction."""

    def _drain_and_barrier(self, tick_clock, wait_clock):
        nc = self.nc
        nop_inst = nc.sync.nop()
        wait_clock.add_sem_waits(
            nop_inst.ins, ScopedClock({None: tick_clock.global_clock})
        )
        mi = nop_inst.ins
        waits = list(mi.sync_info.on_wait) if (mi.sync_info and mi.sync_info.on_wait) else []
        if mi.sync_info is not None:
            mi.sync_info.on_wait = []
        assert self.sems is not None
        sem_by_id = {s.num: s for s in self.sems.allocated().values()}
        for w in waits:
            sem = sem_by_id.get(w.id)
            assert sem is not None, f"no sem handle for wait {w}"
            nc.sync.wait_ge(sem, w.wait_value)
        nc.sync.drain()

        nc.all_engine_barrier()
        popped = nc._tile_sem_poison_stack.pop()
        assert popped is self._sem_poison
        nc.clear_and_free_semaphores(list(self.sems.allocated().values()))
        nc.all_engine_barrier()


def _emit_layernorm_T(nc, xT, g_t, b_t, outT, ones_col_f32, ones_col_bf,
                      ones_row_f32, sq_pool, tmp_pool, stat_pool, bcast_pool,
                      small_pool, eps_t):
    """LayerNorm over features of a transposed activation.

    xT:   SBUF [128, 8, 512] f32   (feature-major; feature f = 128*t + p)
    outT: SBUF [128, 8, 512] bf16  normalized * g + b
    Row stats come from ones-vector matmuls (partition+tile reduction in one
    PSUM chain); A=rstd / B=-mean*rstd are broadcast to 128 partitions with a
    rank-1 ones matmul and applied as (x*A + B) * g + b.  The [1,512] scratch
    vectors rotate through 4 pool slots (a [1,N] tile still costs N bytes on
    every partition, so slots are kept few).
    """
    ps_sum = stat_pool.tile([1, R], F32, tag="st_sum", name="ps_sum")
    ps_sq = stat_pool.tile([1, R], F32, tag="st_sq", name="ps_sq")
    for ti in range(8):
        sq_t = sq_pool.tile([128, R], BF16, tag="sq", name="sq_t")
        nc.scalar.activation(out=sq_t[:], in_=xT[:, ti, :], func=AF.Square)
        nc.tensor.matmul(ps_sum[:], ones_col_f32[:], xT[:, ti, :],
                         start=(ti == 0), stop=(ti == 7))
        nc.tensor.matmul(ps_sq[:], ones_col_bf[:], sq_t[:],
                         start=(ti == 0), stop=(ti == 7))
    mean = small_pool.tile([1, R], F32, tag="s0", name="mean")
    nc.vector.tensor_single_scalar(out=mean[:], in_=ps_sum[:], scalar=1.0 / E, op=OP.mult)
    ex2 = small_pool.tile([1, R], F32, tag="s1", name="ex2")
    nc.vector.tensor_single_scalar(out=ex2[:], in_=ps_sq[:], scalar=1.0 / E, op=OP.mult)
    msq = small_pool.tile([1, R], F32, tag="s2", name="msq")
    nc.vector.tensor_mul(out=msq[:], in0=mean[:], in1=mean[:])
    var = small_pool.tile([1, R], F32, tag="s3", name="var")
    nc.vector.tensor_sub(out=var[:], in0=ex2[:], in1=msq[:])
    lnv = small_pool.tile([1, R], F32, tag="s1", name="lnv")
    nc.scalar.activation(out=lnv[:], in_=var[:], func=AF.Ln, bias=eps_t[0:1, :])
    A = small_pool.tile([1, R], F32, tag="s2", name="A")
    nc.scalar.activation(out=A[:], in_=lnv[:], func=AF.Exp, scale=-0.5)
    Bv = small_pool.tile([1, R], F32, tag="s3", name="Bv")
    nc.vector.scalar_tensor_tensor(out=Bv[:], in0=mean[:], scalar=-1.0, in1=A[:],
                                   op0=OP.mult, op1=OP.mult)
    ps_A = bcast_pool.tile([128, R], F32, tag="bc_A", name="ps_A")
    ps_B = bcast_pool.tile([128, R], F32, tag="bc_B", name="ps_B")
    nc.tensor.matmul(ps_A[:], ones_row_f32[:], A[:], start=True, stop=True)
    nc.tensor.matmul(ps_B[:], ones_row_f32[:], Bv[:], start=True, stop=True)
    for ti in range(8):
        tmp = tmp_pool.tile([128, R], F32, tag="ln_tmp", name="tmp")
        nc.vector.tensor_mul(out=tmp[:], in0=xT[:, ti, :], in1=ps_A[:])
        tmp2 = tmp_pool.tile([128, R], F32, tag="ln_tmp2", name="tmp2")
        nc.vector.tensor_add(out=tmp2[:], in0=tmp[:], in1=ps_B[:])
        nc.scalar.activation(out=outT[:, ti, :], in_=tmp2[:], func=AF.Identity,
                             scale=g_t[:, ti:ti + 1], bias=b_t[:, ti:ti + 1])


def _split_sync_waits(nc, maxw=1):
    """Walrus rejects instructions carrying more than a couple of sync waits
    ("Too many sync wait commands"). Move excess waits onto NoOp instructions
    inserted just before, on the same engine queue — semantically identical
    (the engine blocks on the NoOp's wait before reaching the instruction)."""
    cnt = 0
    for f in nc.m.functions:
        for bb in f.blocks:
            insts = bb.instructions
            new_list = []
            for inst in insts:
                si = inst.sync_info
                waits = list(si.on_wait) if (si and si.on_wait) else []
                if len(waits) > maxw:
                    extra, keep = waits[:-maxw], waits[-maxw:]
                    si.on_wait = keep
                    for w in extra:
                        cnt += 1
                        nop = mybir.InstNoOp(
                            name=f"waitsplit-{cnt}", ins=[], outs=[],
                            engine=inst.engine,
                            sync_info=mybir.SyncInfo(on_wait=[w], on_update=[]))
                        new_list.append(nop)
                new_list.append(inst)
            insts[:] = new_list
    return cnt


def build_program():
    nc = bass.Bass("TRN2", target_bir_lowering=False, debug=False,
                   num_devices=N_CORES)

    # ---- DRAM parameters (per-core views, host-prepped) ----
    d_tgtT = nc.declare_dram_parameter("tgtT", [E, R], F32, isOutput=False)
    d_memT = nc.declare_dram_parameter("memT", [E, LK], BF16, isOutput=False)
    d_wq = nc.declare_dram_parameter("wq", [E, E], BF16, isOutput=False)
    d_wk = nc.declare_dram_parameter("wk", [E, E], BF16, isOutput=False)
    d_wva = nc.declare_dram_parameter("wva", [E, H * HW], BF16, isOutput=False)
    d_wo = nc.declare_dram_parameter("wo", [E, E], BF16, isOutput=False)
    d_w1 = nc.declare_dram_parameter("w1", [E, F], BF16, isOutput=False)
    d_w2 = nc.declare_dram_parameter("w2", [F, E], BF16, isOutput=False)
    d_bq = nc.declare_dram_parameter("bqp", [128, 8], F32, isOutput=False)
    d_bk = nc.declare_dram_parameter("bkp", [128, 8], F32, isOutput=False)
    d_bva = nc.declare_dram_parameter("bvap", [1, H * HW], BF16, isOutput=False)
    d_bo = nc.declare_dram_parameter("bop", [128, 8], F32, isOutput=False)
    d_b1 = nc.declare_dram_parameter("b1p", [128, 32], F32, isOutput=False)
    d_b2 = nc.declare_dram_parameter("b2p", [128, 8], F32, isOutput=False)
    d_g1 = nc.declare_dram_parameter("g1p", [128, 8], F32, isOutput=False)
    d_bb1 = nc.declare_dram_parameter("bb1p", [128, 8], F32, isOutput=False)
    d_g3 = nc.declare_dram_parameter("g3p", [128, 8], F32, isOutput=False)
    d_bb3 = nc.declare_dram_parameter("bb3p", [128, 8], F32, isOutput=False)
    d_outT = nc.declare_dram_parameter("outT", [E, R], F32, isOutput=True)
    d_attnT = nc.declare_dram_parameter("attnT", [LK, R], F32, isOutput=True)
    d_cscr = nc.dram_tensor("cscratch", [H, R], BF16)

    with PatchedTileContext(nc) as tc, ExitStack() as top:
        consts = top.enter_context(tc.tile_pool(name="consts", bufs=1))
        persist = top.enter_context(tc.tile_pool(name="persist", bufs=1))

        # ---- constants / small parameter tiles ----
        ones_col_f32 = consts.tile([128, 1], F32)
        nc.vector.memset(ones_col_f32[:], 1.0)
        ones_col_bf = consts.tile([128, 1], BF16)
        nc.vector.memset(ones_col_bf[:], 1.0)
        ones_row_f32 = consts.tile([1, 128], F32)
        nc.vector.memset(ones_row_f32[:], 1.0)
        ones_sq_bf = consts.tile([128, 128], BF16)
        nc.vector.memset(ones_sq_bf[:], 1.0)
        eps_t = consts.tile([128, 1], F32)
        nc.vector.memset(eps_t[:], EPS)
        mln16_t = consts.tile([128, 1], F32)
        nc.vector.memset(mln16_t[:], -math.log(16.0))

        bq_t = consts.tile([128, 8], F32)
        bk_t = consts.tile([128, 8], F32)
        bo_t = consts.tile([128, 8], F32)
        b1_t = consts.tile([128, 32], F32)
        b2_t = consts.tile([128, 8], F32)
        g1_t = consts.tile([128, 8], F32)
        bb1_t = consts.tile([128, 8], F32)
        g3_t = consts.tile([128, 8], F32)
        bb3_t = consts.tile([128, 8], F32)
        bva_t = consts.tile([1, H * HW], BF16)
        for dst, src in ((bq_t, d_bq), (bk_t, d_bk), (bo_t, d_bo),
                         (b1_t, d_b1), (b2_t, d_b2), (g1_t, d_g1),
                         (bb1_t, d_bb1), (g3_t, d_g3), (bb3_t, d_bb3),
                         (bva_t, d_bva)):
            nc.gpsimd.dma_start(out=dst[:], in_=src[:])

        xT = persist.tile([128, 8, R], F32)

        # ===== attention-era tensors (live through phase 2) =====
        attn_stack = ExitStack()
        actp = attn_stack.enter_context(tc.tile_pool(name="actp", bufs=1))
        projpA = attn_stack.enter_context(tc.tile_pool(name="projpA", bufs=2, space="PSUM"))
        memT = actp.tile([128, 8, LK], BF16, tag="memT", name="memT")
        nc.scalar.dma_start(out=memT[:], in_=d_memT[:].rearrange("(t p) k -> p t k", p=128))
        wk_t = actp.tile([128, 8, E], BF16, tag="wk", name="wk_t")
        nc.scalar.dma_start(out=wk_t[:], in_=d_wk[:].rearrange("(t p) o -> p t o", p=128))
        wva_t = actp.tile([128, 8, H * HW], BF16, tag="wva", name="wva_t")
        nc.scalar.dma_start(out=wva_t[:], in_=d_wva[:].rearrange("(t p) o -> p t o", p=128))
        qT = actp.tile([128, 8, R], BF16, tag="qT", name="qT")
        kT = actp.tile([128, 8, LK], BF16, tag="kT", name="kT")
        v_t = actp.tile([128, 8, H * HW], BF16, tag="vT", name="v_t")

        # =========== Phase 1: LN1 + Q projection ===========
        qkv = ExitStack()
        with qkv:
            lnp = qkv.enter_context(tc.tile_pool(name="lnp", bufs=1))
            sqp = qkv.enter_context(tc.tile_pool(name="sqp", bufs=2))
            tmpp = qkv.enter_context(tc.tile_pool(name="tmpp", bufs=2))
            smallp = qkv.enter_context(tc.tile_pool(name="smallp", bufs=1))
            statp = qkv.enter_context(tc.tile_pool(name="statp", bufs=1, space="PSUM"))
            bcp = qkv.enter_context(tc.tile_pool(name="bcp", bufs=1, space="PSUM"))

            tgtT = lnp.tile([128, 8, R], F32, tag="tgtT", name="tgtT")
            _tgt_src = d_tgtT[:].rearrange("(t p) r -> p t r", p=128)
            for _c in range(4):
                nc.sync.dma_start(out=tgtT[:, 2 * _c:2 * _c + 2, :],
                                  in_=_tgt_src[:, 2 * _c:2 * _c + 2, :])
            wq_t = lnp.tile([128, 8, E], BF16, tag="wq", name="wq_t")
            nc.sync.dma_start(out=wq_t[:], in_=d_wq[:].rearrange("(t p) o -> p t o", p=128))

            t2T = lnp.tile([128, 8, R], BF16, tag="t2T", name="t2T")
            _emit_layernorm_T(nc, tgtT, g1_t, bb1_t, t2T, ones_col_f32,
                              ones_col_bf, ones_row_f32, sqp, tmpp, statp,
                              bcp, smallp, eps_t)

            for to in range(8):
                ps = projpA.tile([128, R], F32, tag="proj", name="ps")
                for ti in range(8):
                    nc.tensor.matmul(ps[:], wq_t[:, ti, to * 128:(to + 1) * 128],
                                     t2T[:, ti, :], start=(ti == 0), stop=(ti == 7))
                nc.scalar.activation(out=qT[:, to, :], in_=ps[:], func=AF.Identity,
                                     bias=bq_t[:, to:to + 1])

        # phase 2-3 tensors on the right-side SBUF stack (opened late so the
        # LN1-era left-stack peak stays under budget)
        ph3 = ExitStack()
        ph3p = ph3.enter_context(tc.tile_pool(name="ph3p", bufs=1, side="right"))
        attn_oT = ph3p.tile([128, 8, R], BF16, tag="attn_oT", name="attn_oT")
        wo_t = ph3p.tile([128, 8, E], BF16, tag="wo", name="wo_t")
        nc.scalar.dma_start(out=wo_t[:], in_=d_wo[:].rearrange("(t p) o -> p t o", p=128))

        CH = H * HW // 4  # 260 cols = 4 heads per v chunk

        def emit_kproj(to):
            for kc in range(2):
                ps = projpA.tile([128, R], F32, tag="proj", name="ps")
                for ti in range(8):
                    nc.tensor.matmul(ps[:], wk_t[:, ti, to * 128:(to + 1) * 128],
                                     memT[:, ti, kc * 512:(kc + 1) * 512],
                                     start=(ti == 0), stop=(ti == 7))
                nc.scalar.activation(out=kT[:, to, kc * 512:(kc + 1) * 512],
                                     in_=ps[:], func=AF.Identity,
                                     bias=bk_t[:, to:to + 1])

        def emit_vchunk(ch):
            for kt in range(8):
                ps = projpA.tile([128, CH], F32, tag="proj", name="ps")
                for ti in range(8):
                    nc.tensor.matmul(ps[:], memT[:, ti, kt * 128:(kt + 1) * 128],
                                     wva_t[:, ti, ch * CH:(ch + 1) * CH],
                                     start=(ti == 0), stop=False)
                nc.tensor.matmul(ps[:], ones_sq_bf[0:1, :],
                                 bva_t[:, ch * CH:(ch + 1) * CH],
                                 start=False, stop=True)
                nc.vector.tensor_copy(out=v_t[:, kt, ch * CH:(ch + 1) * CH], in_=ps[:])

        emit_kproj(0)
        emit_kproj(1)
        emit_vchunk(0)

        # =========== Phase 2: attention (16 heads, K/V interleaved) ===========
        att = ExitStack()
        with att:
            accp = att.enter_context(tc.tile_pool(name="accp", bufs=1))
            acc = [accp.tile([128, 2 * R], BF16, tag=f"acc{kp}", name=f"acc{kp}")
                   for kp in range(4)]
            expp = att.enter_context(tc.tile_pool(name="expp", bufs=8))
            cbp = att.enter_context(tc.tile_pool(name="cbp", bufs=3))
            dnp = att.enter_context(tc.tile_pool(name="dnp", bufs=3))
            odtp = att.enter_context(tc.tile_pool(name="odtp", bufs=2))
            scps = att.enter_context(tc.tile_pool(name="scps", bufs=2, space="PSUM"))
            outps = att.enter_context(tc.tile_pool(name="outps", bufs=2, space="PSUM"))

            for g in range(4):
                for h in range(4 * g, 4 * g + 4):
                    ti, off = h // 2, (h % 2) * 64
                    exp_ts = []
                    for ktp in range(4):
                        s_ps = scps.tile([128, 2 * R], F32, tag="sc", name="s_ps")
                        for j in range(2):
                            kt = 2 * ktp + j
                            nc.tensor.matmul(
                                s_ps[:, j * R:(j + 1) * R],
                                kT[off:off + 64, ti, kt * 128:(kt + 1) * 128],
                                qT[off:off + 64, ti, :],
                                start=True, stop=True)
                        e_t = expp.tile([128, 2 * R], BF16, tag="exp", name="e_t")
                        nc.scalar.activation(out=e_t[:], in_=s_ps[:], func=AF.Exp,
                                             scale=SCALE)
                        exp_ts.append(e_t)

                    o_ps = outps.tile([128, R], F32, tag="o", name="o_ps")
                    for kt in range(8):
                        nc.tensor.matmul(o_ps[0:HW, :],
                                         v_t[:, kt, h * HW:(h + 1) * HW],
                                         exp_ts[kt // 2][:, (kt % 2) * R:(kt % 2 + 1) * R],
                                         start=(kt == 0), stop=(kt == 7))

                    # c = 1/(16*denom) via ln->exp (denom lives on partition 64)
                    dn_t = dnp.tile([128, R], F32, tag="dn", name="dn_t")
                    nc.scalar.activation(out=dn_t[64:65, :], in_=o_ps[64:65, :], func=AF.Ln)
                    c_t = dnp.tile([128, R], BF16, tag="c", name="c_t")
                    nc.scalar.activation(out=c_t[64:65, :], in_=dn_t[64:65, :], func=AF.Exp,
                                         scale=-1.0, bias=mln16_t[64:65, :])
                    # broadcast c to all partitions: bounce through DRAM with
                    # a stride-0 partition source AP (idle DMA engines; frees
                    # PSUM banks vs a ones-matmul broadcast). Explicit dep
                    # edges order the loads behind the store.
                    cb = cbp.tile([128, 2 * R], BF16, tag="cbt", name="cb")
                    st = nc.gpsimd.dma_start(out=d_cscr[h:h + 1, :], in_=c_t[64:65, :])
                    _sb = d_cscr[h:h + 1, :]
                    bc_ap = bass.AP(tensor=_sb.tensor, offset=_sb.offset,
                                    ap=[[0, 128], [1, R]])
                    ld1 = nc.gpsimd.dma_start(out=cb[:, 0:R], in_=bc_ap)
                    add_dep_helper(ld1.ins, st.ins, sync=True, reason="cb bcast after store")
                    ld2 = nc.gpsimd.dma_start(out=cb[:, R:2 * R], in_=bc_ap)
                    add_dep_helper(ld2.ins, st.ins, sync=True, reason="cb bcast after store")

                    # normalized per-head attention output rows (x16 undoes /16)
                    if off == 0:
                        nc.vector.scalar_tensor_tensor(
                            out=attn_oT[0:64, ti, :], in0=o_ps[0:64, :], scalar=16.0,
                            in1=cb[0:64, 0:R], op0=OP.mult, op1=OP.mult)
                    else:
                        od_t = odtp.tile([64, R], BF16, tag="od", name="od_t")
                        nc.vector.scalar_tensor_tensor(
                            out=od_t[:], in0=o_ps[0:64, :], scalar=16.0,
                            in1=cb[0:64, 0:R], op0=OP.mult, op1=OP.mult)
                        nc.gpsimd.dma_start(out=attn_oT[64:128, ti, :], in_=od_t[:])

                    # head-mean accumulation, [128,1024] per op
                    for ktp in range(4):
                        if h == 0:
                            nc.vector.tensor_mul(out=acc[ktp][:], in0=exp_ts[ktp][:], in1=cb[:])
                        else:
                            tmp = cbp.tile([128, 2 * R], BF16, tag="acctmp", name="tmp")
                            nc.vector.tensor_mul(out=tmp[:], in0=exp_ts[ktp][:], in1=cb[:])
                            nc.vector.tensor_add(out=acc[ktp][:], in0=acc[ktp][:], in1=tmp[:])

                # K/V projections for the next head group fill PE gaps while
                # ACT/DVE digest this group's softmax work
                if g < 3:
                    emit_kproj(2 * g + 2)
                    emit_kproj(2 * g + 3)
                    emit_vchunk(g + 1)

            # attn output store (bf16 -> f32 cast in DMA)
            for kp in range(4):
                nc.gpsimd.dma_start(out=d_attnT[(2 * kp) * 128:(2 * kp + 1) * 128, :],
                                    in_=acc[kp][:, 0:R])
                nc.gpsimd.dma_start(out=d_attnT[(2 * kp + 1) * 128:(2 * kp + 2) * 128, :],
                                    in_=acc[kp][:, R:2 * R])

        attn_stack.close()

        # =========== Phase 3: out-proj + residual ===========
        p3 = ExitStack()
        with p3:
            tg2p = p3.enter_context(tc.tile_pool(name="tg2p", bufs=1))
            opsp = p3.enter_context(tc.tile_pool(name="ops", bufs=2, space="PSUM"))
            tgt2 = tg2p.tile([128, 8, R], F32, tag="tgt2", name="tgt2")
            nc.sync.dma_start(out=tgt2[:], in_=d_tgtT[:].rearrange("(t p) r -> p t r", p=128))
            for to in range(8):
                ps = opsp.tile([128, R], F32, tag="op", name="ps")
                for ti in range(8):
                    nc.tensor.matmul(ps[:], wo_t[:, ti, to * 128:(to + 1) * 128],
                                     attn_oT[:, ti, :], start=(ti == 0), stop=(ti == 7))
                nc.vector.scalar_tensor_tensor(
                    out=xT[:, to, :], in0=ps[:], scalar=bo_t[:, to:to + 1],
                    in1=tgt2[:, to, :], op0=OP.add, op1=OP.add)
        ph3.close()

        # =========== Phase 4: LN3 + FFN ===========
        t3_stack = ExitStack()
        t3p = t3_stack.enter_context(tc.tile_pool(name="t3p", bufs=1))
        t3T = t3p.tile([128, 8, R], BF16)
        ffn = ExitStack()
        with ffn:
            w1p = ffn.enter_context(tc.tile_pool(name="w1p", bufs=2))
            w2p = ffn.enter_context(tc.tile_pool(name="w2p", bufs=1))
            htp = ffn.enter_context(tc.tile_pool(name="htp", bufs=1))
            fout = ffn.enter_context(tc.tile_pool(name="fout", bufs=3))
            ffnp = ffn.enter_context(tc.tile_pool(name="ffnp", bufs=4, space="PSUM"))

            # w2 rides the ACT HWDGE ring so the w1 chunk loads (SP ring)
            # are not queued behind this 8MB transfer
            w2_t = w2p.tile([128, 32, E], BF16, tag="w2", name="w2_t")
            nc.scalar.dma_start(out=w2_t[:], in_=d_w2[:].rearrange("(t p) o -> p t o", p=128))

            ln3 = ExitStack()
            with ln3:
                sqp4 = ln3.enter_context(tc.tile_pool(name="sqp4", bufs=2))
                tmpp4 = ln3.enter_context(tc.tile_pool(name="tmpp4", bufs=2))
                smallp4 = ln3.enter_context(tc.tile_pool(name="smallp4", bufs=1))
                statp4 = ln3.enter_context(tc.tile_pool(name="statp4", bufs=1, space="PSUM"))
                bcp4 = ln3.enter_context(tc.tile_pool(name="bcp4", bufs=1, space="PSUM"))
                _emit_layernorm_T(nc, xT, g3_t, bb3_t, t3T, ones_col_f32,
                                  ones_col_bf, ones_row_f32, sqp4, tmpp4, statp4,
                                  bcp4, smallp4, eps_t)

            hT = htp.tile([128, 32, R], BF16)
            for fog in range(8):
                w1c = w1p.tile([128, 8, R], BF16, tag="w1c", name="w1c")
                src = d_w1[:, fog * 512:(fog + 1) * 512]
                nc.sync.dma_start(out=w1c[:], in_=src.rearrange("(t p) f -> p t f", p=128))
                for f2 in range(4):
                    fo = fog * 4 + f2
                    ps = ffnp.tile([128, R], F32, tag="ffn", name="ps")
                    for ti in range(8):
                        nc.tensor.matmul(ps[:], w1c[:, ti, f2 * 128:(f2 + 1) * 128],
                                         t3T[:, ti, :], start=(ti == 0), stop=(ti == 7))
                    nc.scalar.activation(out=hT[:, fo, :], in_=ps[:], func=AF.Relu,
                                         bias=b1_t[:, fo:fo + 1])

            for eo in range(8):
                ps = ffnp.tile([128, R], F32, tag="ffn", name="ps")
                for fi in range(32):
                    nc.tensor.matmul(ps[:], w2_t[:, fi, eo * 128:(eo + 1) * 128],
                                     hT[:, fi, :], start=(fi == 0), stop=(fi == 31))
                fo_t = fout.tile([128, R], F32, tag="fo", name="fo_t")
                nc.vector.scalar_tensor_tensor(
                    out=fo_t[:], in0=ps[:], scalar=b2_t[:, eo:eo + 1],
                    in1=xT[:, eo, :], op0=OP.add, op1=OP.add)
                nc.sync.dma_start(out=d_outT[eo * 128:(eo + 1) * 128, :], in_=fo_t[:])
        t3_stack.close()
    _split_sync_waits(nc, maxw=1)
    return nc


_NC = None


def _get_program():
    global _NC
    if _NC is None:
        _NC = build_program()
    return _NC


def kernel(tgt, memory, ln1_g, ln1_b, wq, bq, wk, bk, wv, bv, wo, bo,
           ln3_g, ln3_b, w1, b1, w2, b2):
    tgt = np.asarray(tgt, np.float32)
    memory = np.asarray(memory, np.float32)

    def part_tiles(vec, n):
        # [n*128] bias -> [128, n] per-partition tiles (feature f = 128*t + p)
        return np.ascontiguousarray(np.asarray(vec, np.float32).reshape(n, 128).T)

    wq_b = np.ascontiguousarray(np.asarray(wq, np.float32)).astype(BF)
    wk_b = np.ascontiguousarray(np.asarray(wk, np.float32)).astype(BF)
    wo_b = np.ascontiguousarray(np.asarray(wo, np.float32)).astype(BF)
    w1_b = np.ascontiguousarray(np.asarray(w1, np.float32)).astype(BF)
    w2_b = np.ascontiguousarray(np.asarray(w2, np.float32)).astype(BF)
    # augmented v-projection: per head 64 value cols + 1 zero col whose bias is 1
    wva = np.zeros((E, H * HW), np.float32)
    bva = np.zeros((1, H * HW), np.float32)
    wv_f = np.asarray(wv, np.float32)
    bv_f = np.asarray(bv, np.float32)
    for h in range(H):
        wva[:, h * HW:h * HW + DH] = wv_f[:, h * DH:(h + 1) * DH]
        bva[0, h * HW:h * HW + DH] = bv_f[h * DH:(h + 1) * DH]
        bva[0, h * HW + DH] = 1.0
    wva_b = wva.astype(BF)
    bva_b = bva.astype(BF)

    shared = {
        "wq": wq_b, "wk": wk_b, "wva": wva_b, "wo": wo_b,
        "w1": w1_b, "w2": w2_b,
        "bqp": part_tiles(bq, 8), "bkp": part_tiles(bk, 8),
        "bvap": bva_b, "bop": part_tiles(bo, 8),
        "b1p": part_tiles(b1, 32), "b2p": part_tiles(b2, 8),
        "g1p": part_tiles(ln1_g, 8), "bb1p": part_tiles(ln1_b, 8),
        "g3p": part_tiles(ln3_g, 8), "bb3p": part_tiles(ln3_b, 8),
    }
    in_maps = []
    for c in range(N_CORES):
        b, hh = c // 2, c % 2
        rows = tgt[b, hh * R:(hh + 1) * R]            # [512, 1024]
        m = {"tgtT": np.ascontiguousarray(rows.T),
             "memT": np.ascontiguousarray(memory[b].T.astype(BF))}
        m.update(shared)
        in_maps.append(m)

    nc = _get_program()
    res = run_bass_kernel_spmd(nc, in_maps, list(range(N_CORES)))

    out = np.empty((B, LQ, E), np.float32)
    attn = np.empty((B, LQ, LK), np.float32)
    for c in range(N_CORES):
        b, hh = c // 2, c % 2
        out[b, hh * R:(hh + 1) * R] = res.results[c]["outT"].T
        attn[b, hh * R:(hh + 1) * R] = res.results[c]["attnT"].T
    return out, attn
